# revision 34
# baseline (speedup 1.0000x reference)
"""NodeGraphContrastiveLoss on 8 Trainium2 cores.

loss = mean_n[ ln(rowsum_n - exp(pos_n)) - pos_n ],  pos_n = cos(l_n, g_{n//128})/T,
rowsum_n = sum_k exp(cos(l_n, g_k)/T).

Host folds 1/(T*||l_n||) into l (so device dots ARE cos/T), quantizes both
operands to fp8e4, and computes the positive-pair dots + final log/mean
itself.  The device only produces rowsum.

Device layout is k-on-partitions ("transposed"): per 512-row block, 8
k-chunks of the similarity matrix are computed as fp8 DoubleRow matmuls
(256-deep contraction in one MM), exponentiated into fp8 codes by one of
two engines, then summed over k by DoubleRow "ones" matmuls that
accumulate every block's rowsums into a single PSUM bank ([32 blocks x 512
rows]), which is DMA'd out once at the end.

exp engines (split tuned to the cost model):
  A: ScalarE activation Exp (exact, fp8 out)     ~1.04 us / unit
  D: DVE Schraudolph bit-trick from PSUM         ~1.19 us / unit
where a unit = 2 k-chunks x 512 rows = [128, 2, 512] dots.

Schraudolph-to-fp8: fp8e4 bits of exp(x) ~ int8(x * 8/ln2 + 55.55); the
bias constant is calibrated to zero the mean error for x ~ N(0, 0.31)
(the actual cos/T distribution) under round-to-nearest int8 conversion,
so rowsums are unbiased to ~2e-4.
"""

import numpy as np
import ml_dtypes
from contextlib import ExitStack

import concourse.bass as bass
import concourse.tile as tile
from concourse import bacc, mybir
from concourse.bass_utils import run_bass_kernel_spmd

T = 0.2
N_CORES = 8
B, A, C, K = 1024, 128, 256, 1024
N = B * A                  # 131072 rows total
NL = N // N_CORES          # 16384 rows per core
R = 512                    # rows per block
NBLK = NL // R             # 32 blocks per core
UNIT_CHUNKS = (2, 2, 2, 2)  # k-chunk grouping per block (8 chunks of 128)
NUNIT = len(UNIT_CHUNKS)    # exp units per block
PSUM_BUFS = 3               # 3 x 2-bank psum tiles + 1 acc bank = 7 of 8
FP8NP = ml_dtypes.float8_e4m3

F32 = mybir.dt.float32
I8 = mybir.dt.int8
FP8 = mybir.dt.float8e4
AF = mybir.ActivationFunctionType
ALU = mybir.AluOpType
DR = mybir.MatmulPerfMode.DoubleRow

# Schraudolph exp -> fp8e4 bit trick constants (see module docstring).
# B8 is calibrated for round-to-nearest f32->int8 conversion (the DVE
# rounds; numpy's astype truncates, which would need +0.5).
A8 = float(8.0 / np.log(2.0))
B8 = 55.55

# exp-engine assignment per unit (128 units/core): counts tuned so each
# engine's busy time is ~equal under the TRN2 cost model.  (No GPS units:
# DMA cannot read PSUM in this bass, so GpSimd cannot be fed affordably.)
N_ACT, N_DVE = 64, 64


def _unit_pattern():
    """Interleave A/D units evenly across the 128 units."""
    counts = {"D": N_DVE, "A": N_ACT}
    total = sum(counts.values())
    assert total == NBLK * NUNIT
    err = {t: 0.0 for t in counts}
    pat = []
    for _ in range(total):
        for t in err:
            err[t] += counts[t] / total
        pick = max(err, key=lambda t: err[t])
        err[pick] -= 1.0
        pat.append(pick)
    return pat


PATTERN = _unit_pattern()
ONES_LAG = 4  # emit each ones-MM late to avoid PE head-of-line stalls
DEBUG_NO_ONES = False  # timing ablation: skip ones-MMs (wrong results)
DEBUG_NO_EXP = False   # timing ablation: skip exp instructions (wrong results)

LAST_RESULTS = None  # BassKernelResults of the most recent run (for test.py)
_NC = None


def _build():
    nc = bacc.Bacc(None, target_bir_lowering=False)
    # lt[b, ki, ko, r] = l_scaled_fp8[block b row r, channel ko*128+ki]
    lt = nc.dram_tensor("lt", [NBLK, 128, 2, R], FP8, kind="ExternalInput")
    # g[ki, ko, k] = ghat_fp8[k, channel ko*128+ki]
    g = nc.dram_tensor("g", [128, 2, K], FP8, kind="ExternalInput")
    # ow[ki, ko, j, c] = 1.0 if c == j else 0  (ones column for block j)
    ow = nc.dram_tensor("ow", [128, 2, NBLK, 32], FP8, kind="ExternalInput")
    rs = nc.dram_tensor("rs", [NBLK, R], F32, kind="ExternalOutput")

    with tile.TileContext(nc) as tc, ExitStack() as ctx:
        singles = ctx.enter_context(tc.tile_pool(name="singles", bufs=1))
        lt_pool = ctx.enter_context(tc.tile_pool(name="ltp", bufs=3))
        e8_pool = ctx.enter_context(tc.tile_pool(name="e8p", bufs=ONES_LAG + 3))
        psum = ctx.enter_context(tc.tile_pool(name="psum", bufs=PSUM_BUFS, space="PSUM"))
        psacc = ctx.enter_context(tc.tile_pool(name="psacc", bufs=1, space="PSUM"))

        gh = singles.tile([128, 2, K], FP8)
        # split so the first unit's weights (chunks 0-1) land early
        nc.sync.dma_start(out=gh[:, :, 0:256], in_=g[:, :, 0:256])
        lt0 = lt_pool.tile([128, 2, R], FP8, tag="lt")
        nc.sync.dma_start(out=lt0[:], in_=lt[0])
        nc.sync.dma_start(out=gh[:, :, 256:K], in_=g[:, :, 256:K])
        onesw = singles.tile([128, 2, NBLK, 32], FP8)
        nc.sync.dma_start(out=onesw[:], in_=ow[:, :, :, :])
        acc = psacc.tile([128, R], F32)

        pending = []  # (block, e8_tile, n_chunks) awaiting their ones-MMs
        emitted = 0

        n_ones = sum(nch // 2 + nch % 2 for nch in UNIT_CHUNKS) * NBLK

        def emit_ones(blk, e8t, nch):
            nonlocal emitted
            if DEBUG_NO_ONES:
                return
            for j in range(0, nch - 1, 2):
                nc.tensor.matmul(
                    acc[0:32, :],
                    onesw[:, :, blk, :],
                    e8t[:, j:j + 2, :],
                    start=(emitted == 0),
                    stop=(emitted == n_ones - 1),
                    perf_mode=DR,
                    skip_group_check=True,
                )
                emitted += 1
            if nch % 2:
                nc.tensor.matmul(
                    acc[0:32, :],
                    onesw[:, 0, blk, :],
                    e8t[:, nch - 1, :],
                    start=(emitted == 0),
                    stop=(emitted == n_ones - 1),
                    skip_group_check=True,
                )
                emitted += 1

        ui = 0
        for b in range(NBLK):
            if b == 0:
                ltb = lt0
            else:
                ltb = lt_pool.tile([128, 2, R], FP8, tag="lt")
                nc.sync.dma_start(out=ltb[:], in_=lt[b])
            cs = 0
            for nch in UNIT_CHUNKS:
                ty = PATTERN[ui]
                ps = psum.tile([128, nch, R], F32, tag="ps")
                for i in range(nch):
                    ck = cs + i
                    nc.tensor.matmul(
                        ps[:, i, :],
                        gh[:, :, ck * 128:(ck + 1) * 128],
                        ltb[:, :, :],
                        start=True,
                        stop=True,
                        perf_mode=DR,
                        skip_group_check=True,
                    )
                e8 = e8_pool.tile([128, nch, R], FP8, tag="e8")
                if DEBUG_NO_EXP:
                    pass
                elif ty == "A":
                    nc.scalar.activation(out=e8[:], in_=ps[:], func=AF.Exp)
                else:
                    nc.vector.tensor_scalar(
                        out=e8[:].bitcast(I8), in0=ps[:],
                        scalar1=A8, scalar2=B8, op0=ALU.mult, op1=ALU.add,
                    )
                pending.append((b, e8, nch))
                if len(pending) > ONES_LAG:
                    emit_ones(*pending.pop(0))
                cs += nch
                ui += 1
        while pending:
            emit_ones(*pending.pop(0))

        rs_sb = singles.tile([32, R], F32)
        nc.vector.tensor_copy(out=rs_sb[:], in_=acc[0:32, :])
        nc.sync.dma_start(out=rs[:, :], in_=rs_sb[:])
    nc.finalize()
    return nc


def _get_nc():
    global _NC
    if _NC is None:
        _NC = _build()
    return _NC


def _make_onesw():
    w = np.zeros((128, 2, NBLK, 32), dtype=FP8NP)
    for j in range(NBLK):
        w[:, :, j, j] = FP8NP(1.0)
    return w


def _prep_core(lq, i):
    rows = lq[i * NL:(i + 1) * NL]                     # [16384, 256] fp8
    arr = rows.reshape(NBLK, R, 2, 128)                # [b, r, ko, ki]
    arr = np.ascontiguousarray(arr.transpose(0, 3, 2, 1))  # [b, ki, ko, r]
    return arr


def kernel(l_enc, g_enc, **run_kwargs):
    global LAST_RESULTS
    l2 = np.asarray(l_enc, dtype=np.float32).reshape(N, C)
    ge = np.asarray(g_enc, dtype=np.float32)

    norms = np.linalg.norm(l2, axis=1, keepdims=True)
    lq = (l2 / (T * norms)).astype(FP8NP)              # [N, C] fp8
    gq = (ge / np.linalg.norm(ge, axis=1, keepdims=True)).astype(FP8NP)

    garr = np.ascontiguousarray(
        gq.astype(FP8NP).T.reshape(2, 128, K).transpose(1, 0, 2))  # [ki, ko, k]
    onesw = _make_onesw()

    in_maps = [
        {"lt": _prep_core(lq, i), "g": garr, "ow": onesw} for i in range(N_CORES)
    ]
    nc = _get_nc()
    res = run_bass_kernel_spmd(nc, in_maps, core_ids=list(range(N_CORES)), **run_kwargs)
    LAST_RESULTS = res

    # positive-pair dots from the same quantized operands the device used
    lqf = lq.astype(np.float32)
    gqf = gq.astype(np.float32)
    pos = np.einsum("bac,bc->ba", lqf.reshape(B, A, C), gqf).reshape(N)
    pos = pos.astype(np.float64)

    rowsum = np.concatenate(
        [np.asarray(r["rs"], dtype=np.float64).reshape(NL) for r in res.results])
    loss = np.mean(np.log(rowsum - np.exp(pos)) - pos)
    return np.float32(loss)


# revision 47
# speedup vs baseline: 1.0132x; 1.0132x over previous
"""NodeGraphContrastiveLoss on 8 Trainium2 cores.

loss = mean_n[ ln(rowsum_n - exp(pos_n)) - pos_n ],  pos_n = cos(l_n, g_{n//128})/T,
rowsum_n = sum_k exp(cos(l_n, g_k)/T).

Host folds 1/(T*||l_n||) into l (so device dots ARE cos/T), quantizes both
operands to fp8e4, and computes the positive-pair dots + final log/mean
itself.  The device only produces rowsum.

Device layout is k-on-partitions ("transposed"): per 512-row block, 8
k-chunks of the similarity matrix are computed as fp8 DoubleRow matmuls
(256-deep contraction in one MM), exponentiated into fp8 codes, then summed
over k by DoubleRow "ones" matmuls that accumulate every block's rowsums
into a single PSUM bank ([32 blocks x 512 rows]), DMA'd out once at the end.

The exp work runs as TWO INDEPENDENT single-engine pipelines (a shared
ring would lock both engines to the slower one's pace):
  ACT stream: 18 blocks, units of 2 k-chunks, psum ring 2 x 2 banks,
              ScalarE activation Exp (exact, fp8 out), ~1.05 ns/elem
  DVE stream: 14 blocks, units of 1 k-chunk,  psum ring 3 x 1 bank,
              DVE Schraudolph bit-trick exp,          ~1.35 ns/elem
plus 1 PSUM bank for the rowsum accumulator = 8 banks total.

Schraudolph-to-fp8: fp8e4 bits of exp(x) ~ int8(x * 8/ln2 + 55.55); the
bias constant is calibrated to zero the mean error for x ~ N(0, 0.31)
(the actual cos/T distribution) under round-to-nearest int8 conversion,
so rowsums are unbiased to ~2e-4.
"""

import numpy as np
import ml_dtypes
from contextlib import ExitStack

import concourse.bass as bass
import concourse.tile as tile
from concourse import bacc, mybir
from concourse.bass_utils import run_bass_kernel_spmd

T = 0.2
N_CORES = 8
B, A, C, K = 1024, 128, 256, 1024
N = B * A                  # 131072 rows total
NL = N // N_CORES          # 16384 rows per core
R = 512                    # rows per block
NBLK = NL // R             # 32 blocks per core
NCH = 8                    # k-chunks of 128 per block
FP8NP = ml_dtypes.float8_e4m3

F32 = mybir.dt.float32
I8 = mybir.dt.int8
FP8 = mybir.dt.float8e4
AF = mybir.ActivationFunctionType
ALU = mybir.AluOpType
DR = mybir.MatmulPerfMode.DoubleRow

# Schraudolph exp -> fp8e4 bit trick constants (see module docstring).
A8 = float(8.0 / np.log(2.0))
B8 = 55.55

# blocks per stream (tuned so both engines finish together under the
# TRN2 cost model: ACT ~4.29us/block, DVE ~5.52us/block)
N_ACT_BLK, N_DVE_BLK = 18, 14

# approx wall time per unit, for merge-by-virtual-time emission
T_UNIT_ACT = 1073.0   # 2-chunk activation unit
T_UNIT_DVE = 690.0    # 1-chunk tensor_scalar unit

ACT_LAG = 2   # ACT-stream ones-MM lag (units)
DVE_LAG = 1   # DVE-stream ones-MM lag (pairs)
# trailing chunks of the last DVE block handled by the ACT stream as one
# extra 2-chunk unit (fine-grained pole balancing); 0 or 2.  Measured: 0
# is better (the ACT pole is already the longer one in situ).
TAIL_CHUNKS = 0

LAST_RESULTS = None  # BassKernelResults of the most recent run (for test.py)
_NC = None


def _block_stream():
    """Assign blocks to streams, interleaved evenly (True = ACT)."""
    counts = {True: N_ACT_BLK, False: N_DVE_BLK}
    total = N_ACT_BLK + N_DVE_BLK
    assert total == NBLK
    err = {t: 0.0 for t in counts}
    out = []
    for _ in range(total):
        for t in err:
            err[t] += counts[t] / total
        pick = max(err, key=lambda t: err[t])
        err[pick] -= 1.0
        out.append(pick)
    return out


def _build():
    nc = bacc.Bacc(None, target_bir_lowering=False)
    # lt[b, ki, ko, r] = l_scaled_fp8[block b row r, channel ko*128+ki]
    lt = nc.dram_tensor("lt", [NBLK, 128, 2, R], FP8, kind="ExternalInput")
    # g[ki, ko, k] = ghat_fp8[k, channel ko*128+ki]
    g = nc.dram_tensor("g", [128, 2, K], FP8, kind="ExternalInput")
    # ow[ki, ko, j, c] = 1.0 if c == j else 0  (ones column for block j)
    ow = nc.dram_tensor("ow", [128, 2, NBLK, 32], FP8, kind="ExternalInput")
    rs = nc.dram_tensor("rs", [NBLK, R], F32, kind="ExternalOutput")

    streams = _block_stream()
    d_blocks_all = [b for b in range(NBLK) if not streams[b]]
    tail_blk = d_blocks_all[-1]  # ACT takes this block's last TAIL_CHUNKS
    # TAIL_CHUNKS moves one ones-MM from the DVE to the ACT stream; the
    # total count is unchanged.
    n_ones = (N_ACT_BLK + N_DVE_BLK) * (NCH // 2)

    with tile.TileContext(nc) as tc, ExitStack() as ctx:
        singles = ctx.enter_context(tc.tile_pool(name="singles", bufs=1))
        lt_pool = ctx.enter_context(tc.tile_pool(name="ltp", bufs=4))
        e8a_pool = ctx.enter_context(tc.tile_pool(name="e8a", bufs=4))
        e8d_pool = ctx.enter_context(tc.tile_pool(name="e8d", bufs=3))
        psum_a = ctx.enter_context(tc.tile_pool(name="psA", bufs=2, space="PSUM"))
        psum_d = ctx.enter_context(tc.tile_pool(name="psD", bufs=3, space="PSUM"))
        psacc = ctx.enter_context(tc.tile_pool(name="psacc", bufs=1, space="PSUM"))

        gh = singles.tile([128, 2, K], FP8)
        # split so the first units' weights (chunks 0-1) land early
        nc.sync.dma_start(out=gh[:, :, 0:256], in_=g[:, :, 0:256])
        lt0 = lt_pool.tile([128, 2, R], FP8, tag="lt")
        nc.sync.dma_start(out=lt0[:], in_=lt[0])
        nc.sync.dma_start(out=gh[:, :, 256:K], in_=g[:, :, 256:K])
        onesw = singles.tile([128, 2, NBLK, 32], FP8)
        nc.sync.dma_start(out=onesw[:], in_=ow[:, :, :, :])
        acc = psacc.tile([128, R], F32)

        emitted = 0

        def emit_ones(blk, e8t):
            nonlocal emitted
            rhs = e8t[:, :, :]
            if e8t.dtype == I8:
                rhs = rhs.bitcast(FP8)
            nc.tensor.matmul(
                acc[0:32, :],
                onesw[:, :, blk, :],
                rhs,
                start=(emitted == 0),
                stop=(emitted == n_ones - 1),
                perf_mode=DR,
                skip_group_check=True,
            )
            emitted += 1

        # Per-stream generators: each yields one unit of work per next() and
        # manages its own psum/e8 rings and lagged ones-MMs.
        def act_unit(blk, ck0, pending):
            ps = psum_a.tile([128, 2, R], F32, tag="psa")
            for i in range(2):
                ck = ck0 + i
                nc.tensor.matmul(
                    ps[:, i, :],
                    gh[:, :, ck * 128:(ck + 1) * 128],
                    lt_tiles[blk][:, :, :],
                    start=True, stop=True,
                    perf_mode=DR, skip_group_check=True,
                )
            e8 = e8a_pool.tile([128, 2, R], FP8, tag="e8a")
            nc.scalar.activation(out=e8[:], in_=ps[:], func=AF.Exp)
            pending.append((blk, e8))
            if len(pending) > ACT_LAG:
                emit_ones(*pending.pop(0))

        def act_stream():
            pending = []
            for blk in (b for b in range(NBLK) if streams[b]):
                for p in range(NCH // 2):
                    act_unit(blk, p * 2, pending)
                    yield T_UNIT_ACT
            if TAIL_CHUNKS:
                act_unit(tail_blk, NCH - 2, pending)
                yield T_UNIT_ACT
            while pending:
                emit_ones(*pending.pop(0))

        def dve_stream():
            # e8 tiles are int8 (the Schraudolph bits); the ones-MM reads
            # them bitcast to fp8e4.
            pending = []
            for bi, blk in enumerate(b for b in range(NBLK) if not streams[b]):
                ltb = lt_tiles[blk]
                e8 = None
                nch = NCH - (TAIL_CHUNKS if blk == tail_blk else 0)
                for ck in range(nch):
                    ps = psum_d.tile([128, R], F32, tag="psd")
                    nc.tensor.matmul(
                        ps[:, :],
                        gh[:, :, ck * 128:(ck + 1) * 128],
                        ltb[:, :, :],
                        start=True, stop=True,
                        perf_mode=DR, skip_group_check=True,
                    )
                    if ck % 2 == 0:
                        e8 = e8d_pool.tile([128, 2, R], I8, tag="e8d")
                    nc.vector.tensor_scalar(
                        out=e8[:, ck % 2, :], in0=ps[:],
                        scalar1=A8, scalar2=B8, op0=ALU.mult, op1=ALU.add,
                    )
                    if ck % 2 == 1:
                        pending.append((blk, e8))
                        if len(pending) > DVE_LAG:
                            emit_ones(*pending.pop(0))
                    yield T_UNIT_DVE
            while pending:
                emit_ones(*pending.pop(0))

        # lt DMAs: issued in global block order just before a block's first
        # unit; the streams read their tiles from this dict.
        lt_tiles = {0: lt0}
        next_lt = 1

        def issue_lt_upto(blk):
            nonlocal next_lt
            while next_lt <= blk:
                t_ = lt_pool.tile([128, 2, R], FP8, tag="lt")
                nc.sync.dma_start(out=t_[:], in_=lt[next_lt])
                lt_tiles[next_lt] = t_
                next_lt += 1

        # issue lt DMAs in global block order, interleaved with unit emission:
        # wrap the generators so that before a block's first unit, its lt DMA
        # (and all earlier blocks') has been issued.
        def wrap(gen_blocks, gen):
            it = iter(gen)
            bidx = 0
            per_block = {True: NCH // 2, False: NCH}
            while True:
                if bidx < len(gen_blocks):
                    issue_lt_upto(gen_blocks[bidx])
                n = per_block[streams[gen_blocks[bidx]]] if bidx < len(gen_blocks) else 0
                for _ in range(max(n, 1)):
                    try:
                        yield next(it)
                    except StopIteration:
                        return
                bidx += 1

        a_blocks = [b for b in range(NBLK) if streams[b]]
        d_blocks = [b for b in range(NBLK) if not streams[b]]
        a_it = wrap(a_blocks, act_stream())
        d_it = wrap(d_blocks, dve_stream())

        ta = td = 0.0
        a_done = d_done = False
        while not (a_done and d_done):
            if d_done or (not a_done and ta <= td):
                try:
                    ta += next(a_it)
                except StopIteration:
                    a_done = True
            else:
                try:
                    td += next(d_it)
                except StopIteration:
                    d_done = True

        rs_sb = singles.tile([32, R], F32)
        nc.scalar.activation(out=rs_sb[:], in_=acc[0:32, :], func=AF.Copy)
        nc.sync.dma_start(out=rs[:, :], in_=rs_sb[:])
    nc.finalize()
    return nc


def _get_nc():
    global _NC
    if _NC is None:
        _NC = _build()
    return _NC


def _make_onesw():
    w = np.zeros((128, 2, NBLK, 32), dtype=FP8NP)
    for j in range(NBLK):
        w[:, :, j, j] = FP8NP(1.0)
    return w


def _prep_core(lq, i):
    rows = lq[i * NL:(i + 1) * NL]                     # [16384, 256] fp8
    arr = rows.reshape(NBLK, R, 2, 128)                # [b, r, ko, ki]
    arr = np.ascontiguousarray(arr.transpose(0, 3, 2, 1))  # [b, ki, ko, r]
    return arr


def kernel(l_enc, g_enc, **run_kwargs):
    global LAST_RESULTS
    l2 = np.asarray(l_enc, dtype=np.float32).reshape(N, C)
    ge = np.asarray(g_enc, dtype=np.float32)

    norms = np.linalg.norm(l2, axis=1, keepdims=True)
    lq = (l2 / (T * norms)).astype(FP8NP)              # [N, C] fp8
    gq = (ge / np.linalg.norm(ge, axis=1, keepdims=True)).astype(FP8NP)

    garr = np.ascontiguousarray(
        gq.astype(FP8NP).T.reshape(2, 128, K).transpose(1, 0, 2))  # [ki, ko, k]
    onesw = _make_onesw()

    in_maps = [
        {"lt": _prep_core(lq, i), "g": garr, "ow": onesw} for i in range(N_CORES)
    ]
    nc = _get_nc()
    res = run_bass_kernel_spmd(nc, in_maps, core_ids=list(range(N_CORES)), **run_kwargs)
    LAST_RESULTS = res

    # positive-pair dots from the same quantized operands the device used
    lqf = lq.astype(np.float32)
    gqf = gq.astype(np.float32)
    pos = np.einsum("bac,bc->ba", lqf.reshape(B, A, C), gqf).reshape(N)
    pos = pos.astype(np.float64)

    rowsum = np.concatenate(
        [np.asarray(r["rs"], dtype=np.float64).reshape(NL) for r in res.results])
    loss = np.mean(np.log(rowsum - np.exp(pos)) - pos)
    return np.float32(loss)


# revision 52
# speedup vs baseline: 1.0150x; 1.0018x over previous
"""NodeGraphContrastiveLoss on 8 Trainium2 cores.

loss = mean_n[ ln(rowsum_n - exp(pos_n)) - pos_n ],  pos_n = cos(l_n, g_{n//128})/T,
rowsum_n = sum_k exp(cos(l_n, g_k)/T).

Host folds 1/(T*||l_n||) into l (so device dots ARE cos/T), quantizes both
operands to fp8e4, and computes the positive-pair dots + final log/mean
itself.  The device only produces rowsum.

Device layout is k-on-partitions ("transposed"): per 512-row block, 8
k-chunks of the similarity matrix are computed as fp8 DoubleRow matmuls
(256-deep contraction in one MM), exponentiated into fp8 codes, then summed
over k by DoubleRow "ones" matmuls that accumulate every block's rowsums
into a single PSUM bank ([32 blocks x 512 rows]), DMA'd out once at the end.

The exp work runs as TWO INDEPENDENT single-engine pipelines (a shared
ring would lock both engines to the slower one's pace):
  ACT stream: 18 blocks, units of 2 k-chunks, psum ring 2 x 2 banks,
              ScalarE activation Exp (exact, fp8 out), ~1.05 ns/elem
  DVE stream: 14 blocks, units of 1 k-chunk,  psum ring 3 x 1 bank,
              DVE Schraudolph bit-trick exp,          ~1.35 ns/elem
plus 1 PSUM bank for the rowsum accumulator = 8 banks total.

Schraudolph-to-fp8: fp8e4 bits of exp(x) ~ int8(x * 8/ln2 + 55.55); the
bias constant is calibrated to zero the mean error for x ~ N(0, 0.31)
(the actual cos/T distribution) under round-to-nearest int8 conversion,
so rowsums are unbiased to ~2e-4.
"""

import numpy as np
import ml_dtypes
from contextlib import ExitStack

import concourse.bass as bass
import concourse.tile as tile
from concourse import bacc, mybir
from concourse.bass_utils import run_bass_kernel_spmd

T = 0.2
N_CORES = 8
B, A, C, K = 1024, 128, 256, 1024
N = B * A                  # 131072 rows total
NL = N // N_CORES          # 16384 rows per core
R = 512                    # rows per block
NBLK = NL // R             # 32 blocks per core
NCH = 8                    # k-chunks of 128 per block
FP8NP = ml_dtypes.float8_e4m3

F32 = mybir.dt.float32
I8 = mybir.dt.int8
FP8 = mybir.dt.float8e4
AF = mybir.ActivationFunctionType
ALU = mybir.AluOpType
DR = mybir.MatmulPerfMode.DoubleRow

# Schraudolph exp -> fp8e4 bit trick constants (see module docstring).
A8 = float(8.0 / np.log(2.0))
B8 = 55.55

# blocks per stream (tuned so both engines finish together under the
# TRN2 cost model: ACT ~4.29us/block, DVE ~5.52us/block)
N_ACT_BLK, N_DVE_BLK = 18, 14

# approx wall time per unit, for merge-by-virtual-time emission (tuned:
# the schedule is sensitive to this ratio; 1097/703 is the sweep optimum)
T_UNIT_ACT = 1097.0   # 2-chunk activation unit
T_UNIT_DVE = 703.0    # 1-chunk tensor_scalar unit

ACT_LAG = 2   # ACT-stream ones-MM lag (units)
DVE_LAG = 1   # DVE-stream ones-MM lag (pairs)
# trailing chunks of the last DVE block handled by the ACT stream as one
# extra 2-chunk unit (fine-grained pole balancing); 0 or 2.  Measured: 0
# is better (the ACT pole is already the longer one in situ).
TAIL_CHUNKS = 0

LAST_RESULTS = None  # BassKernelResults of the most recent run (for test.py)
_NC = None


def _block_stream():
    """Assign blocks to streams, interleaved evenly (True = ACT)."""
    counts = {True: N_ACT_BLK, False: N_DVE_BLK}
    total = N_ACT_BLK + N_DVE_BLK
    assert total == NBLK
    err = {t: 0.0 for t in counts}
    out = []
    for _ in range(total):
        for t in err:
            err[t] += counts[t] / total
        pick = max(err, key=lambda t: err[t])
        err[pick] -= 1.0
        out.append(pick)
    return out


def _build():
    nc = bacc.Bacc(None, target_bir_lowering=False)
    # lt[b, ki, ko, r] = l_scaled_fp8[block b row r, channel ko*128+ki]
    lt = nc.dram_tensor("lt", [NBLK, 128, 2, R], FP8, kind="ExternalInput")
    # g[ki, ko, k] = ghat_fp8[k, channel ko*128+ki]
    g = nc.dram_tensor("g", [128, 2, K], FP8, kind="ExternalInput")
    # ow[ki, ko, j, c] = 1.0 if c == j else 0  (ones column for block j)
    ow = nc.dram_tensor("ow", [128, 2, NBLK, 32], FP8, kind="ExternalInput")
    rs = nc.dram_tensor("rs", [NBLK, R], F32, kind="ExternalOutput")

    streams = _block_stream()
    d_blocks_all = [b for b in range(NBLK) if not streams[b]]
    tail_blk = d_blocks_all[-1]  # ACT takes this block's last TAIL_CHUNKS
    # TAIL_CHUNKS moves one ones-MM from the DVE to the ACT stream; the
    # total count is unchanged.
    n_ones = (N_ACT_BLK + N_DVE_BLK) * (NCH // 2)

    with tile.TileContext(nc) as tc, ExitStack() as ctx:
        singles = ctx.enter_context(tc.tile_pool(name="singles", bufs=1))
        lt_pool = ctx.enter_context(tc.tile_pool(name="ltp", bufs=4))
        e8a_pool = ctx.enter_context(tc.tile_pool(name="e8a", bufs=4))
        e8d_pool = ctx.enter_context(tc.tile_pool(name="e8d", bufs=3))
        psum_a = ctx.enter_context(tc.tile_pool(name="psA", bufs=2, space="PSUM"))
        psum_d = ctx.enter_context(tc.tile_pool(name="psD", bufs=3, space="PSUM"))
        psacc = ctx.enter_context(tc.tile_pool(name="psacc", bufs=1, space="PSUM"))

        gh = singles.tile([128, 2, K], FP8)
        # split so the first units' weights (chunks 0-1) land early; issue
        # the startup DMAs from different sequencers so they overlap
        nc.sync.dma_start(out=gh[:, :, 0:256], in_=g[:, :, 0:256])
        lt0 = lt_pool.tile([128, 2, R], FP8, tag="lt")
        nc.scalar.dma_start(out=lt0[:], in_=lt[0])
        nc.sync.dma_start(out=gh[:, :, 256:K], in_=g[:, :, 256:K])
        onesw = singles.tile([128, 2, NBLK, 32], FP8)
        nc.sync.dma_start(out=onesw[:], in_=ow[:, :, :, :])
        acc = psacc.tile([128, R], F32)

        emitted = 0

        def emit_ones(blk, e8t):
            nonlocal emitted
            rhs = e8t[:, :, :]
            if e8t.dtype == I8:
                rhs = rhs.bitcast(FP8)
            nc.tensor.matmul(
                acc[0:32, :],
                onesw[:, :, blk, :],
                rhs,
                start=(emitted == 0),
                stop=(emitted == n_ones - 1),
                perf_mode=DR,
                skip_group_check=True,
            )
            emitted += 1

        # Per-stream generators: each yields one unit of work per next() and
        # manages its own psum/e8 rings and lagged ones-MMs.
        def act_unit(blk, ck0, pending):
            ps = psum_a.tile([128, 2, R], F32, tag="psa")
            for i in range(2):
                ck = ck0 + i
                nc.tensor.matmul(
                    ps[:, i, :],
                    gh[:, :, ck * 128:(ck + 1) * 128],
                    lt_tiles[blk][:, :, :],
                    start=True, stop=True,
                    perf_mode=DR, skip_group_check=True,
                )
            e8 = e8a_pool.tile([128, 2, R], FP8, tag="e8a")
            nc.scalar.activation(out=e8[:], in_=ps[:], func=AF.Exp)
            pending.append((blk, e8))
            if len(pending) > ACT_LAG:
                emit_ones(*pending.pop(0))

        def act_stream():
            pending = []
            for blk in (b for b in range(NBLK) if streams[b]):
                for p in range(NCH // 2):
                    act_unit(blk, p * 2, pending)
                    yield T_UNIT_ACT
            if TAIL_CHUNKS:
                act_unit(tail_blk, NCH - 2, pending)
                yield T_UNIT_ACT
            while pending:
                emit_ones(*pending.pop(0))

        def dve_stream():
            # e8 tiles are int8 (the Schraudolph bits); the ones-MM reads
            # them bitcast to fp8e4.
            pending = []
            for bi, blk in enumerate(b for b in range(NBLK) if not streams[b]):
                ltb = lt_tiles[blk]
                e8 = None
                nch = NCH - (TAIL_CHUNKS if blk == tail_blk else 0)
                for ck in range(nch):
                    ps = psum_d.tile([128, R], F32, tag="psd")
                    nc.tensor.matmul(
                        ps[:, :],
                        gh[:, :, ck * 128:(ck + 1) * 128],
                        ltb[:, :, :],
                        start=True, stop=True,
                        perf_mode=DR, skip_group_check=True,
                    )
                    if ck % 2 == 0:
                        e8 = e8d_pool.tile([128, 2, R], I8, tag="e8d")
                    nc.vector.tensor_scalar(
                        out=e8[:, ck % 2, :], in0=ps[:],
                        scalar1=A8, scalar2=B8, op0=ALU.mult, op1=ALU.add,
                    )
                    if ck % 2 == 1:
                        pending.append((blk, e8))
                        if len(pending) > DVE_LAG:
                            emit_ones(*pending.pop(0))
                    yield T_UNIT_DVE
            while pending:
                emit_ones(*pending.pop(0))

        # lt DMAs: issued in global block order just before a block's first
        # unit; the streams read their tiles from this dict.
        lt_tiles = {0: lt0}
        next_lt = 1

        def issue_lt_upto(blk):
            nonlocal next_lt
            while next_lt <= blk:
                t_ = lt_pool.tile([128, 2, R], FP8, tag="lt")
                nc.sync.dma_start(out=t_[:], in_=lt[next_lt])
                lt_tiles[next_lt] = t_
                next_lt += 1

        # issue lt DMAs in global block order, interleaved with unit emission:
        # wrap the generators so that before a block's first unit, its lt DMA
        # (and all earlier blocks') has been issued.
        def wrap(gen_blocks, gen):
            it = iter(gen)
            bidx = 0
            per_block = {True: NCH // 2, False: NCH}
            while True:
                if bidx < len(gen_blocks):
                    issue_lt_upto(gen_blocks[bidx])
                n = per_block[streams[gen_blocks[bidx]]] if bidx < len(gen_blocks) else 0
                for _ in range(max(n, 1)):
                    try:
                        yield next(it)
                    except StopIteration:
                        return
                bidx += 1

        a_blocks = [b for b in range(NBLK) if streams[b]]
        d_blocks = [b for b in range(NBLK) if not streams[b]]
        a_it = wrap(a_blocks, act_stream())
        d_it = wrap(d_blocks, dve_stream())

        ta = td = 0.0
        a_done = d_done = False
        while not (a_done and d_done):
            if d_done or (not a_done and ta <= td):
                try:
                    ta += next(a_it)
                except StopIteration:
                    a_done = True
            else:
                try:
                    td += next(d_it)
                except StopIteration:
                    d_done = True

        rs_sb = singles.tile([32, R], F32)
        nc.scalar.activation(out=rs_sb[:], in_=acc[0:32, :], func=AF.Copy)
        nc.sync.dma_start(out=rs[:, :], in_=rs_sb[:])
    nc.finalize()
    return nc


def _get_nc():
    global _NC
    if _NC is None:
        _NC = _build()
    return _NC


def _make_onesw():
    w = np.zeros((128, 2, NBLK, 32), dtype=FP8NP)
    for j in range(NBLK):
        w[:, :, j, j] = FP8NP(1.0)
    return w


def _prep_core(lq, i):
    rows = lq[i * NL:(i + 1) * NL]                     # [16384, 256] fp8
    arr = rows.reshape(NBLK, R, 2, 128)                # [b, r, ko, ki]
    arr = np.ascontiguousarray(arr.transpose(0, 3, 2, 1))  # [b, ki, ko, r]
    return arr


def kernel(l_enc, g_enc, **run_kwargs):
    global LAST_RESULTS
    l2 = np.asarray(l_enc, dtype=np.float32).reshape(N, C)
    ge = np.asarray(g_enc, dtype=np.float32)

    norms = np.linalg.norm(l2, axis=1, keepdims=True)
    lq = (l2 / (T * norms)).astype(FP8NP)              # [N, C] fp8
    gq = (ge / np.linalg.norm(ge, axis=1, keepdims=True)).astype(FP8NP)

    garr = np.ascontiguousarray(
        gq.astype(FP8NP).T.reshape(2, 128, K).transpose(1, 0, 2))  # [ki, ko, k]
    onesw = _make_onesw()

    in_maps = [
        {"lt": _prep_core(lq, i), "g": garr, "ow": onesw} for i in range(N_CORES)
    ]
    nc = _get_nc()
    res = run_bass_kernel_spmd(nc, in_maps, core_ids=list(range(N_CORES)), **run_kwargs)
    LAST_RESULTS = res

    # positive-pair dots from the same quantized operands the device used
    lqf = lq.astype(np.float32)
    gqf = gq.astype(np.float32)
    pos = np.einsum("bac,bc->ba", lqf.reshape(B, A, C), gqf).reshape(N)
    pos = pos.astype(np.float64)

    rowsum = np.concatenate(
        [np.asarray(r["rs"], dtype=np.float64).reshape(NL) for r in res.results])
    loss = np.mean(np.log(rowsum - np.exp(pos)) - pos)
    return np.float32(loss)
